# revision 1
# baseline (speedup 1.0000x reference)
"""DeepSeek-style MoE layer (64 routed experts, top-6 grouped routing, 2
shared experts) on 8 Trainium2 NeuronCores.

v2: identical slot-tile structure to the baseline, but routed expert
weights are stored/streamed as fp8-e3m4 (halving the dominant HBM
traffic) with global power-of-2 scales folded into the SiLU activation
scale (gate) and the y psum->sbuf copy (up*down).  Activations are fp16
(PE takes mixed fp16 x fp8 operands; accumulate is fp32 either way).
Output partials are fp16 (summed in fp32 on host).
"""
import numpy as np
import ml_dtypes

import concourse.bacc as bacc
import concourse.mybir as mybir
from concourse import tile
from concourse.bass_utils import run_bass_kernel_spmd

F16 = np.float16
BF16 = ml_dtypes.bfloat16
E3 = ml_dtypes.float8_e3m4
E3MAX = 15.5

T, H, E, I = 1024, 1024, 64, 512
NG, TKG, TOPK = 8, 3, 6
RSF = 2.5
P = 128
KT = H // P          # 8 k-tiles over hidden dim
IT = I // P          # 4 tiles over intermediate dim
HN = H // 512        # 2 output column chunks
TN = T // P          # 8 token tiles
N_CORES = 8
SI_SLICE = P         # shared-expert intermediate slice per core (2*512/8)


def _pow2floor(v):
    return 2.0 ** np.floor(np.log2(v))


# ---------------------------------------------------------------- routing
def _route(x, gate_w, e_bias):
    """Exact fp32 replica of the reference noaux_tc grouped top-k."""
    logits = x.astype(np.float32) @ gate_w.astype(np.float32)
    scores = 1.0 / (1.0 + np.exp(-logits))
    sc = scores + e_bias
    g = sc.reshape(T, NG, E // NG)
    top2 = np.sort(g, axis=-1)[:, :, -2:].sum(-1)
    gidx = np.argsort(-top2, axis=-1)[:, :TKG]
    gmask = np.zeros((T, NG), bool)
    gmask[np.arange(T)[:, None], gidx] = True
    emask = np.repeat(gmask, E // NG, axis=1)
    masked = np.where(emask, sc, -np.inf)
    ids = np.argsort(-masked, axis=-1)[:, :TOPK]
    w = np.take_along_axis(scores, ids, axis=1)
    w = w / w.sum(-1, keepdims=True)
    return ids, w


def _pack(ids, w):
    """Assign exactly E//N_CORES experts to each core (one weight stream per
    expert, no duplicate weight DMA).  Experts with load > P ("big", <= 2P)
    are placed at positions 6/7 of the per-core order; overflow tokens go to
    slot tiles 8/9 which structurally reuse the SBUF weights of positions
    6/7 (wsel below), keeping the module uniform across cores."""
    per_e = []
    for e in range(E):
        rows, cols = np.nonzero(ids == e)
        per_e.append((rows, w[rows, cols] * RSF))
    loads = np.array([len(r) for r, _ in per_e])
    epc = E // N_CORES                      # experts per core (8)
    if loads.max() <= 2 * P and (loads > P).sum() <= 2 * N_CORES:
        bigs = sorted([e for e in range(E) if loads[e] > P],
                      key=lambda e: -loads[e])
        smalls = sorted([e for e in range(E) if loads[e] <= P],
                        key=lambda e: -loads[e])
        core_exp = [[] for _ in range(N_CORES)]
        big_cnt = [0] * N_CORES
        slot_sum = [0] * N_CORES
        for e in bigs:
            c = min((i for i in range(N_CORES)
                     if big_cnt[i] < 2 and len(core_exp[i]) < epc),
                    key=lambda i: (big_cnt[i], slot_sum[i]))
            core_exp[c].append(e)
            big_cnt[c] += 1
            slot_sum[c] += loads[e]
        for e in smalls:
            c = min((i for i in range(N_CORES) if len(core_exp[i]) < epc),
                    key=lambda i: (slot_sum[i], len(core_exp[i])))
            core_exp[c].append(e)
            slot_sum[c] += loads[e]
        ns = epc + 2
        wsel = list(range(epc)) + [epc - 2, epc - 1]
        cores = []
        for c in range(N_CORES):
            exps = core_exp[c]
            order = [e for e in exps if loads[e] <= P] + \
                    [e for e in exps if loads[e] > P]
            order = order[:epc]
            tiles = []
            for s in range(epc):
                e = order[s]
                toks, wts = per_e[e]
                tiles.append((e, toks[:P], wts[:P]))
            for pos in (epc - 2, epc - 1):
                e = order[pos]
                toks, wts = per_e[e]
                tiles.append((e, toks[P:2 * P], wts[P:2 * P]))
            cores.append((order, tiles))
        return cores, ns, epc, wsel
    # fallback: generic tile bin-packing, one weight stream per tile
    tiles = []
    for e in range(E):
        rows, wts = per_e[e]
        for off in range(0, len(rows), P):
            tiles.append((e, rows[off:off + P], wts[off:off + P]))
    cores0 = [[] for _ in range(N_CORES)]
    for t in sorted(tiles, key=lambda z: -len(z[1])):
        c = min(range(N_CORES),
                key=lambda i: (len(cores0[i]), sum(len(z[1]) for z in cores0[i])))
        cores0[c].append(t)
    ns = max(len(c) for c in cores0)
    cores = []
    for c in range(N_CORES):
        ctiles = list(cores0[c])
        while len(ctiles) < ns:
            ctiles.append((0, np.zeros(0, np.int64), np.zeros(0, np.float32)))
        cores.append(([e for e, _, _ in ctiles], ctiles))
    return cores, ns, ns, list(range(ns))


def _prepare(inputs):
    """Host-side shard/dispatch: returns (in_maps, ns, n_wsets, wsel, scales)."""
    x = np.asarray(inputs["hidden_states"], np.float32)
    w_gate = np.asarray(inputs["w_gate"], np.float32)
    w_up = np.asarray(inputs["w_up"], np.float32)
    w_down = np.asarray(inputs["w_down"], np.float32)
    sw_gate = np.asarray(inputs["sw_gate"], np.float32)
    sw_up = np.asarray(inputs["sw_up"], np.float32)
    sw_down = np.asarray(inputs["sw_down"], np.float32)

    ids, w = _route(x, np.asarray(inputs["gate_w"], np.float32),
                    np.asarray(inputs["e_bias"], np.float32))
    cores, ns, n_wsets, wsel = _pack(ids, w)

    # global power-of-two fp8 scales per weight type (uniform across cores
    # so the SPMD module's baked-in descale constants are identical)
    SG = _pow2floor(0.75 * E3MAX / np.abs(w_gate).max())
    SU = _pow2floor(0.75 * E3MAX / np.abs(w_up).max())
    SD = _pow2floor(0.75 * E3MAX / np.abs(w_down).max())

    # xt layout [P, KT*T]: xt[p, k*T + t] = x[t, k*P + p]
    xt = np.ascontiguousarray(
        x.T.reshape(KT, P, T).transpose(1, 0, 2).reshape(P, KT * T)).astype(BF16)
    # pre-reshaped expert weights in sbuf layout, fp8-e3m4 with global scales
    wg_sb_all = np.ascontiguousarray(
        (w_gate * SG).reshape(E, KT, P, I).transpose(0, 2, 1, 3).reshape(E, P, KT * I)).astype(E3)
    wu_sb_all = np.ascontiguousarray(
        (w_up * SU).reshape(E, KT, P, I).transpose(0, 2, 1, 3).reshape(E, P, KT * I)).astype(E3)
    wd_sb_all = np.ascontiguousarray(
        (w_down * SD).reshape(E, IT, P, H).transpose(0, 2, 1, 3).reshape(E, P, IT * H)).astype(E3)

    in_maps = []
    for c in range(N_CORES):
        order, ctiles = cores[c]
        xg = np.zeros((KT, P, ns * P), np.float32)
        st = np.zeros((ns, P, T), F16)
        wg_t = np.zeros((n_wsets, P, KT * I), E3)
        wu_t = np.zeros((n_wsets, P, KT * I), E3)
        wd_t = np.zeros((n_wsets, P, IT * H), E3)
        for j in range(n_wsets):
            e = order[j] if j < len(order) else 0
            wg_t[j] = wg_sb_all[e]
            wu_t[j] = wu_sb_all[e]
            wd_t[j] = wd_sb_all[e]
        for s, (e, toks, wts) in enumerate(ctiles):
            n = len(toks)
            if n:
                xg[:, :, s * P:s * P + n] = x[toks].T.reshape(KT, P, n)
                st[s, np.arange(n), toks] = wts.astype(F16)
        # xg layout [P, KT*ns*P]: xg2[p, k*ns*P + col] = x[tok(col), k*P + p]
        xg2 = np.ascontiguousarray(
            xg.transpose(1, 0, 2).reshape(P, KT * ns * P)).astype(F16)
        # st layout [P, ns*T]
        st2 = np.ascontiguousarray(
            st.transpose(1, 0, 2).reshape(P, ns * T))
        sl = slice(c * SI_SLICE, (c + 1) * SI_SLICE)
        swg = np.ascontiguousarray(
            sw_gate[:, sl].reshape(KT, P, SI_SLICE).transpose(1, 0, 2)
            .reshape(P, KT * SI_SLICE)).astype(BF16)
        swu = np.ascontiguousarray(
            sw_up[:, sl].reshape(KT, P, SI_SLICE).transpose(1, 0, 2)
            .reshape(P, KT * SI_SLICE)).astype(BF16)
        in_maps.append({
            "xg": xg2,
            "st": st2,
            "wg": wg_t,
            "wu": wu_t,
            "wd": wd_t,
            "xt": xt,
            "swg": swg,
            "swu": swu,
            "swd": sw_down[sl, :].astype(BF16),
            "ident": np.eye(P, dtype=BF16),
        })
    return in_maps, ns, n_wsets, wsel, (SG, SU, SD)


# ----------------------------------------------------------------- device
def _build(ns, n_wsets, wsel, scales, loop_n=1, skip_compute=False,
           skip_combine=False):
    """loop_n > 1 wraps the whole body in a device-side loop; used only for
    timing measurements (marginal cost per iteration = true exec time)."""
    import contextlib
    SG, SU, SD = scales
    FP16 = mybir.dt.float16
    E3D = mybir.dt.float8e3
    F32 = mybir.dt.float32
    SILU = mybir.ActivationFunctionType.Silu
    COPY = mybir.ActivationFunctionType.Copy
    BF = mybir.dt.bfloat16

    nc = bacc.Bacc(None, target_bir_lowering=False)
    xg_d = nc.declare_dram_parameter("xg", [P, KT * ns * P], FP16, isOutput=False)
    st_d = nc.declare_dram_parameter("st", [P, ns * T], FP16, isOutput=False)
    wg_d = nc.declare_dram_parameter("wg", [n_wsets, P, KT * I], E3D, isOutput=False)
    wu_d = nc.declare_dram_parameter("wu", [n_wsets, P, KT * I], E3D, isOutput=False)
    wd_d = nc.declare_dram_parameter("wd", [n_wsets, P, IT * H], E3D, isOutput=False)
    xt_d = nc.declare_dram_parameter("xt", [P, KT * T], BF, isOutput=False)
    swg_d = nc.declare_dram_parameter("swg", [P, KT * SI_SLICE], BF, isOutput=False)
    swu_d = nc.declare_dram_parameter("swu", [P, KT * SI_SLICE], BF, isOutput=False)
    swd_d = nc.declare_dram_parameter("swd", [SI_SLICE, H], BF, isOutput=False)
    ident_d = nc.declare_dram_parameter("ident", [P, P], BF, isOutput=False)
    out_d = nc.declare_dram_parameter("out", [T, H], BF, isOutput=True)

    with tile.TileContext(nc) as tc:
        with tc.tile_pool(name="big", bufs=1) as big, \
             tc.tile_pool(name="dbl", bufs=2) as dbl, \
             tc.tile_pool(name="wpool", bufs=4) as wpool, \
             tc.tile_pool(name="hpool", bufs=3) as hpool, \
             tc.tile_pool(name="opool", bufs=6) as opool, \
             tc.tile_pool(name="gup", bufs=3, space="PSUM") as gup, \
             tc.tile_pool(name="ypsum", bufs=2, space="PSUM") as ypsum, \
             tc.tile_pool(name="capool", bufs=3, space="PSUM") as ca, \
             tc.tile_pool(name="oapool", bufs=16) as oapool, \
             (tc.For_i(0, loop_n, 1) if loop_n > 1 else contextlib.nullcontext()):

            xt_sb = big.tile([P, KT * T], BF, tag="xt")
            nc.sync.dma_start(out=xt_sb[:], in_=xt_d[:])
            swg_sb = big.tile([P, KT * SI_SLICE], BF, tag="swg")
            nc.sync.dma_start(out=swg_sb[:], in_=swg_d[:])
            swu_sb = big.tile([P, KT * SI_SLICE], BF, tag="swu")
            nc.sync.dma_start(out=swu_sb[:], in_=swu_d[:])
            swd_sb = big.tile([P, H], BF, tag="swd")
            nc.sync.dma_start(out=swd_sb[:], in_=swd_d[:])
            ident_sb = big.tile([P, P], BF, tag="ident")
            nc.sync.dma_start(out=ident_sb[:], in_=ident_d[:])
            xg_sb = big.tile([P, KT * ns * P], FP16, tag="xg")
            nc.sync.dma_start(out=xg_sb[:], in_=xg_d[:])
            st_sb = dbl.tile([P, ns * T], FP16, tag="st")
            nc.sync.dma_start(out=st_sb[:], in_=st_d[:])
            y_sb = dbl.tile([P, ns * H], FP16, tag="y")
            hsh_sb = big.tile([P, T], BF, tag="hsh")

            # ---- shared experts (TP slice of SI on this core), all fp16
            for tn in range(T // 512 if not skip_compute else 0):
                pg = gup.tile([P, 512], F32, tag="pg")
                for k in range(KT):
                    nc.tensor.matmul(
                        pg[:],
                        swg_sb[:, k * SI_SLICE:(k + 1) * SI_SLICE],
                        xt_sb[:, k * T + tn * 512: k * T + tn * 512 + 512],
                        start=(k == 0), stop=(k == KT - 1))
                hg = hpool.tile([P, 512], F32, tag="hg")
                nc.scalar.activation(hg[:], pg[:], SILU)
                pu = gup.tile([P, 512], F32, tag="pg")
                for k in range(KT):
                    nc.tensor.matmul(
                        pu[:],
                        swu_sb[:, k * SI_SLICE:(k + 1) * SI_SLICE],
                        xt_sb[:, k * T + tn * 512: k * T + tn * 512 + 512],
                        start=(k == 0), stop=(k == KT - 1))
                nc.vector.tensor_mul(hsh_sb[:, tn * 512:(tn + 1) * 512], hg[:], pu[:])

            # ---- routed experts, one slot tile (<=128 tokens, one expert) at
            # a time; tiles with wsel[s] < s reuse already-resident weights
            wtiles = {}
            SPLIT = ns // 2
            oa_tiles = {}
            for s in range(ns):
                j = wsel[s]
                if j not in wtiles:
                    wgs = wpool.tile([P, KT * I], E3D, tag="wg")
                    nc.sync.dma_start(out=wgs[:], in_=wg_d[j])
                    wus = wpool.tile([P, KT * I], E3D, tag="wu")
                    nc.sync.dma_start(out=wus[:], in_=wu_d[j])
                    wds = wpool.tile([P, IT * H], E3D, tag="wd")
                    nc.sync.dma_start(out=wds[:], in_=wd_d[j])
                    wtiles[j] = (wgs, wus, wds)
                else:
                    wgs, wus, wds = wtiles[j]
                if skip_compute:
                    continue

                # gate/up with xg (tokens) stationary, fp8 weights moving
                pg = gup.tile([P, I], F32, tag="pg")
                for k in range(KT):
                    nc.tensor.matmul(
                        pg[:],
                        xg_sb[:, (k * ns + s) * P: (k * ns + s + 1) * P],
                        wgs[:, k * I: (k + 1) * I],
                        start=(k == 0), stop=(k == KT - 1))
                hg = hpool.tile([P, I], F32, tag="hg")
                nc.scalar.activation(hg[:], pg[:], SILU, scale=float(1.0 / SG))
                pu = gup.tile([P, I], F32, tag="pg")
                for k in range(KT):
                    nc.tensor.matmul(
                        pu[:],
                        xg_sb[:, (k * ns + s) * P: (k * ns + s + 1) * P],
                        wus[:, k * I: (k + 1) * I],
                        start=(k == 0), stop=(k == KT - 1))
                hb = hpool.tile([P, I], BF, tag="hb")
                nc.vector.tensor_mul(hb[:], hg[:], pu[:])  # carries SU factor
                pt = ypsum.tile([P, I], BF, tag="py")
                for isl in range(IT):
                    nc.tensor.transpose(
                        pt[:, isl * P:(isl + 1) * P],
                        hb[:, isl * P:(isl + 1) * P],
                        ident_sb[:])
                hbT = hpool.tile([P, I], BF, tag="hbT")
                nc.vector.tensor_copy(out=hbT[:], in_=pt[:])
                for hn in range(HN):
                    py = ypsum.tile([P, 512], F32, tag="py")
                    for isl in range(IT):
                        nc.tensor.matmul(
                            py[:],
                            hbT[:, isl * P:(isl + 1) * P],
                            wds[:, isl * H + hn * 512: isl * H + hn * 512 + 512],
                            start=(isl == 0), stop=(isl == IT - 1))
                    # descale (up*down fp8 scales) on the psum->sbuf copy
                    # (ACT engine: DVE psum-read + fp16-write faults the HW)
                    nc.scalar.activation(
                        y_sb[:, s * H + hn * 512: s * H + hn * 512 + 512],
                        py[:], COPY, scale=float(1.0 / (SU * SD)))
                if s == SPLIT - 1 and not (skip_compute or skip_combine):
                    # group-A combine over tiles 0..SPLIT-1
                    for tm in range(TN):
                        for hn in range(HN):
                            pa = ca.tile([P, 512], F32, tag="ca")
                            for s2 in range(SPLIT):
                                nc.tensor.matmul(
                                    pa[:],
                                    st_sb[:, s2 * T + tm * P: s2 * T + (tm + 1) * P],
                                    y_sb[:, s2 * H + hn * 512: s2 * H + hn * 512 + 512],
                                    start=(s2 == 0), stop=(s2 == SPLIT - 1))
                            oa = oapool.tile([P, 512], FP16, tag="oa")
                            nc.scalar.activation(oa[:], pa[:], COPY)
                            oa_tiles[(tm, hn)] = oa

            # ---- combine: out[T,H] = sum_s ST_s.T @ Y_s  + hsh.T @ swd
            if skip_compute or skip_combine:
                for tm in range(TN):
                    for hn in range(HN):
                        ob = opool.tile([P, 512], BF, tag="ob")
                        nc.any.memset(ob[:], 0.0)
                        nc.sync.dma_start(
                            out=out_d[tm * P:(tm + 1) * P, hn * 512:(hn + 1) * 512],
                            in_=ob[:])
            else:
              for tm in range(TN):
                for hn in range(HN):
                    pc = ca.tile([P, 512], F32, tag="ca")
                    for s in range(SPLIT, ns):
                        nc.tensor.matmul(
                            pc[:],
                            st_sb[:, s * T + tm * P: s * T + (tm + 1) * P],
                            y_sb[:, s * H + hn * 512: s * H + hn * 512 + 512],
                            start=(s == SPLIT), stop=False)
                    nc.tensor.matmul(
                        pc[:],
                        hsh_sb[:, tm * P:(tm + 1) * P],
                        swd_sb[:, hn * 512:(hn + 1) * 512],
                        start=False, stop=True)
                    ob = opool.tile([P, 512], BF, tag="ob")
                    nc.vector.tensor_tensor(
                        out=ob[:], in0=pc[:], in1=oa_tiles[(tm, hn)][:],
                        op=mybir.AluOpType.add)
                    nc.sync.dma_start(
                        out=out_d[tm * P:(tm + 1) * P, hn * 512:(hn + 1) * 512],
                        in_=ob[:])

    nc.finalize()
    return nc


def _run(nc, in_maps):
    res = run_bass_kernel_spmd(nc, in_maps, core_ids=list(range(N_CORES)))
    out = np.zeros((T, H), np.float32)
    for r in res.results:
        out += r["out"].astype(np.float32)
    return out


def kernel(**inputs):
    in_maps, ns, n_wsets, wsel, scales = _prepare(inputs)
    nc = _build(ns, n_wsets, wsel, scales)
    return _run(nc, in_maps)



# revision 30
# speedup vs baseline: 78.8108x; 78.8108x over previous
"""DeepSeek-style MoE layer (64 routed experts, top-6 grouped routing, 2
shared experts) on 8 Trainium2 NeuronCores.

v3: same expert-parallel slot structure as v2 (fp8-e3m4 weight streams,
fp16 activations), but the on-device combine matmuls are eliminated.
The per-token combine weight (and the fp8 descale) is folded into the
down-projection psum->sbuf copy as a per-partition scale, and the
weighted per-slot expert outputs are written directly to DRAM as an
expanded [ns*128, H] tensor.  The host unshard step scatter-adds those
rows into the full output (the same row-gather it already does to
assemble shards).  The shared experts (TP-sharded over the intermediate
dim) keep a dense [T, H] partial per core, summed on host as before.
This removes ~40% of the PE work (16 blocks x (ns+1) combine matmuls)
and the 2.6 MB selection-matrix DMA per core.
"""
import numpy as np
import ml_dtypes

import concourse.bacc as bacc
import concourse.mybir as mybir
from concourse import tile
from concourse.bass_utils import run_bass_kernel_spmd

F16 = np.float16
BF16 = ml_dtypes.bfloat16
E3 = ml_dtypes.float8_e3m4
E3MAX = 15.5

T, H, E, I = 1024, 1024, 64, 512
NG, TKG, TOPK = 8, 3, 6
RSF = 2.5
P = 128
KT = H // P          # 8 k-tiles over hidden dim
IT = I // P          # 4 tiles over intermediate dim
HN = H // 512        # 2 output column chunks
TN = T // P          # 8 token tiles
N_CORES = 8
SI_SLICE = P         # shared-expert intermediate slice per core (2*512/8)


def _pow2floor(v):
    return 2.0 ** np.floor(np.log2(v))


# ---------------------------------------------------------------- routing
def _route(x, gate_w, e_bias):
    """Exact fp32 replica of the reference noaux_tc grouped top-k."""
    logits = x.astype(np.float32) @ gate_w.astype(np.float32)
    scores = 1.0 / (1.0 + np.exp(-logits))
    sc = scores + e_bias
    g = sc.reshape(T, NG, E // NG)
    top2 = np.sort(g, axis=-1)[:, :, -2:].sum(-1)
    gidx = np.argsort(-top2, axis=-1)[:, :TKG]
    gmask = np.zeros((T, NG), bool)
    gmask[np.arange(T)[:, None], gidx] = True
    emask = np.repeat(gmask, E // NG, axis=1)
    masked = np.where(emask, sc, -np.inf)
    ids = np.argsort(-masked, axis=-1)[:, :TOPK]
    w = np.take_along_axis(scores, ids, axis=1)
    w = w / w.sum(-1, keepdims=True)
    return ids, w


def _pack(ids, w):
    """Assign exactly E//N_CORES experts to each core (one weight stream per
    expert, no duplicate weight DMA).  Experts with load > P ("big", <= 2P)
    are placed at positions 6/7 of the per-core order; overflow tokens go to
    slot tiles 8/9 which structurally reuse the SBUF weights of positions
    6/7 (wsel below), keeping the module uniform across cores."""
    per_e = []
    for e in range(E):
        rows, cols = np.nonzero(ids == e)
        per_e.append((rows, w[rows, cols] * RSF))
    loads = np.array([len(r) for r, _ in per_e])
    epc = E // N_CORES                      # experts per core (8)
    if loads.max() <= 2 * P and (loads > P).sum() <= 2 * N_CORES:
        bigs = sorted([e for e in range(E) if loads[e] > P],
                      key=lambda e: -loads[e])
        smalls = sorted([e for e in range(E) if loads[e] <= P],
                        key=lambda e: -loads[e])
        core_exp = [[] for _ in range(N_CORES)]
        big_cnt = [0] * N_CORES
        slot_sum = [0] * N_CORES
        for e in bigs:
            c = min((i for i in range(N_CORES)
                     if big_cnt[i] < 2 and len(core_exp[i]) < epc),
                    key=lambda i: (big_cnt[i], slot_sum[i]))
            core_exp[c].append(e)
            big_cnt[c] += 1
            slot_sum[c] += loads[e]
        for e in smalls:
            c = min((i for i in range(N_CORES) if len(core_exp[i]) < epc),
                    key=lambda i: (slot_sum[i], len(core_exp[i])))
            core_exp[c].append(e)
            slot_sum[c] += loads[e]
        ns = epc + 2
        wsel = list(range(epc)) + [epc - 2, epc - 1]
        cores = []
        for c in range(N_CORES):
            exps = core_exp[c]
            order = [e for e in exps if loads[e] <= P] + \
                    [e for e in exps if loads[e] > P]
            order = order[:epc]
            tiles = []
            for s in range(epc):
                e = order[s]
                toks, wts = per_e[e]
                tiles.append((e, toks[:P], wts[:P]))
            for pos in (epc - 2, epc - 1):
                e = order[pos]
                toks, wts = per_e[e]
                tiles.append((e, toks[P:2 * P], wts[P:2 * P]))
            cores.append((order, tiles))
        return cores, ns, epc, wsel
    # fallback: generic tile bin-packing, one weight stream per tile
    tiles = []
    for e in range(E):
        rows, wts = per_e[e]
        for off in range(0, len(rows), P):
            tiles.append((e, rows[off:off + P], wts[off:off + P]))
    cores0 = [[] for _ in range(N_CORES)]
    for t in sorted(tiles, key=lambda z: -len(z[1])):
        c = min(range(N_CORES),
                key=lambda i: (len(cores0[i]), sum(len(z[1]) for z in cores0[i])))
        cores0[c].append(t)
    ns = max(len(c) for c in cores0)
    cores = []
    for c in range(N_CORES):
        ctiles = list(cores0[c])
        while len(ctiles) < ns:
            ctiles.append((0, np.zeros(0, np.int64), np.zeros(0, np.float32)))
        cores.append(([e for e, _, _ in ctiles], ctiles))
    return cores, ns, ns, list(range(ns))


def _prepare(inputs):
    """Host-side shard/dispatch: returns (in_maps, ns, n_wsets, wsel, scales).
    Each in_map also carries a host-only "tokmap" [ns*P] int32 (the global
    token id of each yout row; padding rows map to 0 with zero weight)."""
    x = np.asarray(inputs["hidden_states"], np.float32)
    w_gate = np.asarray(inputs["w_gate"], np.float32)
    w_up = np.asarray(inputs["w_up"], np.float32)
    w_down = np.asarray(inputs["w_down"], np.float32)
    sw_gate = np.asarray(inputs["sw_gate"], np.float32)
    sw_up = np.asarray(inputs["sw_up"], np.float32)
    sw_down = np.asarray(inputs["sw_down"], np.float32)

    ids, w = _route(x, np.asarray(inputs["gate_w"], np.float32),
                    np.asarray(inputs["e_bias"], np.float32))
    cores, ns, n_wsets, wsel = _pack(ids, w)

    # global power-of-two fp8 scales per weight type (uniform across cores
    # so the SPMD module's baked-in descale constants are identical)
    SG = _pow2floor(0.75 * E3MAX / np.abs(w_gate).max())
    SU = _pow2floor(0.75 * E3MAX / np.abs(w_up).max())
    SD = _pow2floor(0.75 * E3MAX / np.abs(w_down).max())

    # xt layout [P, TN2*KT*512], chunk-major: xt[p, tn*KT*512 + k*512 + t]
    # = x[tn*512 + t, k*P + p] — so each 512-token chunk of the shared
    # expert needs only half of xt (2 half-DMAs overlap the slot phase)
    xt = np.ascontiguousarray(
        x.T.reshape(KT, P, T // 512, 512).transpose(1, 2, 0, 3)
        .reshape(P, KT * T)).astype(BF16)
    # pre-reshaped expert weights in sbuf layout, fp8-e3m4 with global scales
    wg_sb_all = np.ascontiguousarray(
        (w_gate * SG).reshape(E, KT, P, I).transpose(0, 2, 1, 3).reshape(E, P, KT * I)).astype(E3)
    wu_sb_all = np.ascontiguousarray(
        (w_up * SU).reshape(E, KT, P, I).transpose(0, 2, 1, 3).reshape(E, P, KT * I)).astype(E3)
    wd_sb_all = np.ascontiguousarray(
        (w_down * SD).reshape(E, IT, P, H).transpose(0, 2, 1, 3).reshape(E, P, IT * H)).astype(E3)

    # overflow slots epc/epc+1 hold <= 64 tokens each on every core for
    # this routing: run them as one column-tiled pair (slot "epc": tokens
    # 0-63 from the first, 64-127 from the second)
    paired = (ns == E // N_CORES + 2) and all(
        len(cores[c][1][ns - 2][1]) <= 64 and len(cores[c][1][ns - 1][1]) <= 64
        for c in range(N_CORES))
    n_slots = ns - 1 if paired else ns

    in_maps = []
    for c in range(N_CORES):
        order, ctiles = cores[c]
        xg = np.zeros((n_slots, KT, P, P), np.float32)   # [slot, k, p, tok]
        wvec = np.zeros((P, n_slots), np.float32)
        tokmap = np.zeros(n_slots * P, np.int32)
        wg_t = np.zeros((n_wsets, P, KT * I), E3)
        wu_t = np.zeros((n_wsets, P, KT * I), E3)
        wd_t = np.zeros((n_wsets, P, IT * H), E3)
        for j in range(n_wsets):
            e = order[j] if j < len(order) else 0
            wg_t[j] = wg_sb_all[e]
            wu_t[j] = wu_sb_all[e]
            wd_t[j] = wd_sb_all[e]
        wgud_t = np.concatenate([wg_t, wu_t, wd_t], axis=2)  # [n_wsets, P, 3*KT*I]
        for s, (e, toks, wts) in enumerate(ctiles):
            n = len(toks)
            if paired and s >= ns - 2:
                off = (s - (ns - 2)) * 64      # 0 for first, 64 for second
                sp = ns - 2
                if n:
                    xg[sp, :, :, off:off + n] = x[toks].T.reshape(KT, P, n)
                    wvec[off:off + n, sp] = wts / (SU * SD)
                    tokmap[sp * P + off:sp * P + off + n] = toks
            elif n:
                xg[s, :, :, :n] = x[toks].T.reshape(KT, P, n)
                wvec[:n, s] = wts / (SU * SD)
                tokmap[s * P:s * P + n] = toks
        # xg layout [P, ns*KT*P]: xg2[p, (s*KT+k)*P + t] = x[tok(s,t), k*P+p]
        # (slot-major so each slot's activations are one contiguous DMA)
        xg2 = np.ascontiguousarray(
            xg.transpose(2, 0, 1, 3).reshape(P, n_slots * KT * P)).astype(F16)
        sl = slice(c * SI_SLICE, (c + 1) * SI_SLICE)
        swg = np.ascontiguousarray(
            sw_gate[:, sl].reshape(KT, P, SI_SLICE).transpose(1, 0, 2)
            .reshape(P, KT * SI_SLICE)).astype(BF16)
        swu = np.ascontiguousarray(
            sw_up[:, sl].reshape(KT, P, SI_SLICE).transpose(1, 0, 2)
            .reshape(P, KT * SI_SLICE)).astype(BF16)
        in_maps.append({
            "xg": xg2,
            "wgud": wgud_t,
            "wvec": wvec,
            "xt": xt,
            "swg": swg,
            "swu": swu,
            "swd": sw_down[sl, :].astype(BF16),
            "ident": np.eye(P, dtype=BF16),
            "tokmap": tokmap,          # host-only; not a module parameter
        })
    if paired:
        wsel = wsel[:ns - 2] + [(wsel[ns - 2], wsel[ns - 1])]
        ns = n_slots
    return in_maps, ns, n_wsets, wsel, (SG, SU, SD)


# ----------------------------------------------------------------- device
def _build(ns, n_wsets, wsel, scales, loop_n=1):
    """loop_n > 1 wraps the body in a device-side hardware loop; used only
    for timing (marginal cost per iteration = steady-state exec time).  The
    For_i back edge drains every engine, so the kernel software-pipelines
    across it: slot 0/1 weights + activations for iteration i+1 are DMA'd
    during iteration i's second-half DMA slack (a prologue outside the loop
    seeds them for the first iteration)."""
    import contextlib
    SG, SU, SD = scales
    FP16 = mybir.dt.float16
    E3D = mybir.dt.float8e3
    F32 = mybir.dt.float32
    SILU = mybir.ActivationFunctionType.Silu
    COPY = mybir.ActivationFunctionType.Copy
    BF = mybir.dt.bfloat16

    nc = bacc.Bacc(None, target_bir_lowering=False)
    xg_d = nc.declare_dram_parameter("xg", [P, ns * KT * P], FP16, isOutput=False)
    wgud_d = nc.declare_dram_parameter("wgud", [n_wsets, P, 3 * KT * I], E3D, isOutput=False)
    wvec_d = nc.declare_dram_parameter("wvec", [P, ns], F32, isOutput=False)
    xt_d = nc.declare_dram_parameter("xt", [P, KT * T], BF, isOutput=False)
    swg_d = nc.declare_dram_parameter("swg", [P, KT * SI_SLICE], BF, isOutput=False)
    swu_d = nc.declare_dram_parameter("swu", [P, KT * SI_SLICE], BF, isOutput=False)
    swd_d = nc.declare_dram_parameter("swd", [SI_SLICE, H], BF, isOutput=False)
    ident_d = nc.declare_dram_parameter("ident", [P, P], BF, isOutput=False)
    yout_d = nc.declare_dram_parameter("yout", [ns * P, H], BF, isOutput=True)
    out_d = nc.declare_dram_parameter("out", [T, H], BF, isOutput=True)

    GU = 2 * KT * I   # gate+up bytes per wset row; down offset in wgud

    with tile.TileContext(nc) as tc:
        with tc.tile_pool(name="big", bufs=1) as big, \
             tc.tile_pool(name="wpool", bufs=4) as wpool, \
             tc.tile_pool(name="hpool", bufs=3) as hpool, \
             tc.tile_pool(name="ypool", bufs=3) as ypool, \
             tc.tile_pool(name="opool", bufs=2) as opool, \
             tc.tile_pool(name="gup", bufs=3, space="PSUM") as gup, \
             tc.tile_pool(name="ypsum", bufs=3, space="PSUM") as ypsum, \
             tc.tile_pool(name="capool", bufs=2, space="PSUM") as ca:

            # ---- persistent tiles (DMA'd once, outside the loop)
            ident_sb = big.tile([P, P], BF, tag="ident")
            nc.sync.dma_start(out=ident_sb[:], in_=ident_d[:])
            wvec_sb = big.tile([P, ns], F32, tag="wvec")
            nc.sync.dma_start(out=wvec_sb[:], in_=wvec_d[:])

            # ---- skewed (prefetched) tiles for slots 0/1
            pre_w = {}     # wset j -> merged wgud tile
            pre_xg = {}    # slot s -> xg tile
            for idx, s in enumerate((0, 1)):
                j = wsel[s]
                wt = big.tile([P, 3 * KT * I], E3D, tag=f"wgup{idx}")
                pre_w[j] = [wt, wt, GU]
                xgp = big.tile([P, KT * P], FP16, tag=f"xgp{s}")
                pre_xg[s] = xgp

            def emit_prefetch_dmas():
                for idx, s in enumerate((0, 1)):
                    j = wsel[s]
                    nc.sync.dma_start(out=pre_w[j][0][:], in_=wgud_d[j])
                    nc.sync.dma_start(
                        out=pre_xg[s][:],
                        in_=xg_d[:, s * KT * P:(s + 1) * KT * P])

            emit_prefetch_dmas()   # prologue: seed iteration 0

            with (tc.For_i(0, loop_n, 1) if loop_n > 1 else contextlib.nullcontext()):
                wtiles = dict(pre_w)
                xg_tiles = dict(pre_xg)

                def dma_wset_gu(j):
                    wt = wpool.tile([P, GU], E3D, tag="wgu")
                    nc.sync.dma_start(out=wt[:], in_=wgud_d[j][:, :GU])
                    wtiles[j] = [wt, None, 0]

                def dma_wset_d(j):
                    wt = wpool.tile([P, IT * H], E3D, tag="wd")
                    nc.sync.dma_start(out=wt[:], in_=wgud_d[j][:, GU:])
                    wtiles[j][1] = wt

                def dma_xg(s):
                    xgs = big.tile([P, KT * P], FP16, tag=f"xg{s}")
                    nc.sync.dma_start(
                        out=xgs[:], in_=xg_d[:, s * KT * P:(s + 1) * KT * P])
                    xg_tiles[s] = xgs

                # ---- input stream (sync queue), ordered just ahead of use
                dma_wset_gu(wsel[2])
                dma_xg(2)
                dma_wset_d(wsel[2])
                swg_sb = big.tile([P, KT * SI_SLICE], BF, tag="swg")
                nc.sync.dma_start(out=swg_sb[:], in_=swg_d[:])
                swu_sb = big.tile([P, KT * SI_SLICE], BF, tag="swu")
                nc.sync.dma_start(out=swu_sb[:], in_=swu_d[:])
                xt_sb = big.tile([P, KT * T], BF, tag="xt")
                nc.sync.dma_start(out=xt_sb[:, :KT * 512], in_=xt_d[:, :KT * 512])
                dma_wset_gu(wsel[3])
                dma_xg(3)
                dma_wset_d(wsel[3])
                nc.sync.dma_start(out=xt_sb[:, KT * 512:], in_=xt_d[:, KT * 512:])
                swd_sb = big.tile([P, H], BF, tag="swd")
                nc.sync.dma_start(out=swd_sb[:], in_=swd_d[:])
                dma_wset_gu(wsel[4])
                dma_xg(4)
                dma_wset_d(wsel[4])
                dma_xg(5)
                hsh_sb = big.tile([P, T], BF, tag="hsh")

                def slot_body(s, split_out=False):
                    sel = wsel[s]
                    pair = isinstance(sel, tuple)
                    halves = ([(h, wtiles[j]) for h, j in enumerate(sel)]
                              if pair else [(None, wtiles[sel])])
                    xgs = xg_tiles[s]
                    # gate/up with xg (tokens) stationary, fp8 weights moving.
                    # Paired overflow slots (<=64 tokens each) run as two
                    # concurrent column-group matmuls on the PE array.
                    pg = gup.tile([P, I], F32, tag="pg")
                    for k in range(KT):
                        for h, (wgu, _, _) in halves:
                            if h is None:
                                nc.tensor.matmul(
                                    pg[:],
                                    xgs[:, k * P:(k + 1) * P],
                                    wgu[:, k * I: (k + 1) * I],
                                    start=(k == 0), stop=(k == KT - 1))
                            else:
                                nc.tensor.matmul(
                                    pg[64 * h:64 * h + 64, :],
                                    xgs[:, k * P + 64 * h: k * P + 64 * h + 64],
                                    wgu[:, k * I: (k + 1) * I],
                                    start=(k == 0), stop=(k == KT - 1),
                                    tile_position=(0, 64 * h))
                    hg = hpool.tile([P, I], F32, tag="hg")
                    nc.scalar.activation(hg[:], pg[:], SILU, scale=float(1.0 / SG))
                    pu = gup.tile([P, I], F32, tag="pg")
                    for k in range(KT):
                        for h, (wgu, _, _) in halves:
                            if h is None:
                                nc.tensor.matmul(
                                    pu[:],
                                    xgs[:, k * P:(k + 1) * P],
                                    wgu[:, KT * I + k * I: KT * I + (k + 1) * I],
                                    start=(k == 0), stop=(k == KT - 1))
                            else:
                                nc.tensor.matmul(
                                    pu[64 * h:64 * h + 64, :],
                                    xgs[:, k * P + 64 * h: k * P + 64 * h + 64],
                                    wgu[:, KT * I + k * I: KT * I + (k + 1) * I],
                                    start=(k == 0), stop=(k == KT - 1),
                                    tile_position=(0, 64 * h))
                    hb = hpool.tile([P, I], BF, tag="hb")
                    nc.vector.tensor_mul(hb[:], hg[:], pu[:])  # carries SU
                    pt = ypsum.tile([P, I], BF, tag="py")
                    for isl in range(IT):
                        nc.tensor.transpose(
                            pt[:, isl * P:(isl + 1) * P],
                            hb[:, isl * P:(isl + 1) * P],
                            ident_sb[:])
                    hbT = hpool.tile([P, I], BF, tag="hbT")
                    nc.vector.tensor_copy(out=hbT[:], in_=pt[:])
                    ys = ypool.tile([P, H], BF, tag="ys")
                    for hn in range(HN):
                        py = ypsum.tile([P, 512], F32, tag="py")
                        for isl in range(IT):
                            for h, (_, wd, wdb) in halves:
                                if h is None:
                                    nc.tensor.matmul(
                                        py[:],
                                        hbT[:, isl * P:(isl + 1) * P],
                                        wd[:, wdb + isl * H + hn * 512: wdb + isl * H + hn * 512 + 512],
                                        start=(isl == 0), stop=(isl == IT - 1))
                                else:
                                    nc.tensor.matmul(
                                        py[64 * h:64 * h + 64, :],
                                        hbT[:, isl * P + 64 * h: isl * P + 64 * h + 64],
                                        wd[:, wdb + isl * H + hn * 512: wdb + isl * H + hn * 512 + 512],
                                        start=(isl == 0), stop=(isl == IT - 1),
                                        tile_position=(0, 64 * h))
                        # psum->sbuf copy applies combine weight * 1/(SU*SD)
                        # per partition (= per token); on DVE (bf16 write is
                        # the psum-read dtype DVE supports) so the ACT queue
                        # holds only silus + DMA issues
                        nc.vector.tensor_scalar_mul(
                            ys[:, hn * 512:(hn + 1) * 512],
                            py[:], wvec_sb[:, s:s + 1])
                        if split_out:
                            # final slot: DMA each half right after its copy
                            # so the loop-end drain tail is minimal
                            nc.scalar.dma_start(
                                out=yout_d[s * P:(s + 1) * P,
                                           hn * 512:(hn + 1) * 512],
                                in_=ys[:, hn * 512:(hn + 1) * 512])
                    # yout DMA rides the ACT HWDGE queue right behind the
                    # copies that produced ys — it never blocks the weight
                    # stream on the sync queue
                    if not split_out:
                        nc.scalar.dma_start(
                            out=yout_d[s * P:(s + 1) * P, :], in_=ys[:])

                def shared_gate_up(tn):
                    xoff = tn * KT * 512
                    pg = gup.tile([P, 512], F32, tag="pg")
                    for k in range(KT):
                        nc.tensor.matmul(
                            pg[:],
                            swg_sb[:, k * SI_SLICE:(k + 1) * SI_SLICE],
                            xt_sb[:, xoff + k * 512: xoff + (k + 1) * 512],
                            start=(k == 0), stop=(k == KT - 1))
                    hg = hpool.tile([P, 512], F32, tag="hg")
                    nc.scalar.activation(hg[:], pg[:], SILU)
                    pu = gup.tile([P, 512], F32, tag="pg")
                    for k in range(KT):
                        nc.tensor.matmul(
                            pu[:],
                            swu_sb[:, k * SI_SLICE:(k + 1) * SI_SLICE],
                            xt_sb[:, xoff + k * 512: xoff + (k + 1) * 512],
                            start=(k == 0), stop=(k == KT - 1))
                    nc.vector.tensor_mul(hsh_sb[:, tn * 512:(tn + 1) * 512],
                                         hg[:], pu[:])

                def shared_down_block(tm, split_out=False):
                    # one [128, H] block of the shared-expert down projection
                    # (psum->sbuf copies on DVE; DMA issue on the ACT HWDGE
                    # queue).  split_out: per-half DMAs for a minimal drain
                    # tail when this is the last block of the iteration.
                    ob = opool.tile([P, H], BF, tag="ob")
                    for hn in range(HN):
                        pc = ca.tile([P, 512], F32, tag="ca")
                        nc.tensor.matmul(
                            pc[:],
                            hsh_sb[:, tm * P:(tm + 1) * P],
                            swd_sb[:, hn * 512:(hn + 1) * 512],
                            start=True, stop=True)
                        nc.vector.tensor_copy(
                            out=ob[:, hn * 512:(hn + 1) * 512], in_=pc[:])
                        if split_out:
                            nc.scalar.dma_start(
                                out=out_d[tm * P:(tm + 1) * P,
                                          hn * 512:(hn + 1) * 512],
                                in_=ob[:, hn * 512:(hn + 1) * 512])
                    if not split_out:
                        nc.scalar.dma_start(
                            out=out_d[tm * P:(tm + 1) * P, :], in_=ob[:])

                # ---- PE program: slots 0-2, shared gate/up, then the
                # remaining slots with shared-down blocks interleaved; the
                # final shared-down block is split for a minimal drain tail
                slot_body(0)
                if wsel[5] not in wtiles:
                    dma_wset_gu(wsel[5])
                    dma_xg(6)
                    dma_wset_d(wsel[5])
                else:
                    dma_xg(6)
                slot_body(1)
                if wsel[6] not in wtiles:
                    dma_wset_gu(wsel[6])
                    dma_xg(7)
                    dma_wset_d(wsel[6])
                else:
                    dma_xg(7)
                slot_body(2)
                if wsel[7] not in wtiles:
                    dma_wset_gu(wsel[7])
                    dma_wset_d(wsel[7])
                for s in range(8, ns):
                    dma_xg(s)
                if loop_n > 1:
                    emit_prefetch_dmas()   # skew: feed the next iteration
                shared_gate_up(0)
                shared_gate_up(1)
                shared_down_block(0)
                nblk = 1
                for s in range(3, ns):
                    slot_body(s)
                    if nblk < TN - 1:
                        shared_down_block(nblk)
                        nblk += 1
                while nblk < TN - 1:
                    shared_down_block(nblk)
                    nblk += 1
                shared_down_block(TN - 1, split_out=True)

    nc.finalize()
    return nc


def _combine(results, in_maps):
    out = np.zeros((T, H), np.float32)
    for r, m in zip(results, in_maps):
        out += r["out"].astype(np.float32)
        tokmap = m["tokmap"]
        yr = r["yout"].astype(np.float32)
        order = np.argsort(tokmap, kind="stable")
        st = tokmap[order]
        starts = np.nonzero(np.r_[True, st[1:] != st[:-1]])[0]
        seg = np.add.reduceat(yr[order], starts, axis=0)
        out[st[starts]] += seg
    return out


def _run(nc, in_maps):
    res = run_bass_kernel_spmd(nc, in_maps, core_ids=list(range(N_CORES)))
    return _combine(res.results, in_maps)


def kernel(**inputs):
    in_maps, ns, n_wsets, wsel, scales = _prepare(inputs)
    nc = _build(ns, n_wsets, wsel, scales)
    return _run(nc, in_maps)
